# revision 20
# baseline (speedup 1.0000x reference)
"""Trainium2 8-core kernel for biased-attention with sigmoid gating.

Reference computation (per batch b):
  q = heads(q_x @ Wq) * C**-0.5 ; k = heads(kv_x @ Wk) ; v = heads(kv_x @ Wv)
  a = softmax(q k^T + bias1 + bias2, axis=-1)
  o = (a @ v) gated by sigmoid(q_x @ Wg + bg), then @ Wo + bo

Shapes: B=2, Q=K=2048, CQ=CK=CV=256, H=8, C=32, CO=256.

Sharding: 8 cores = 2 batches x 4 query-quarters (512 rows each). Each core
computes all 8 heads for its rows; no cross-core communication is needed.

Design: the dominant HBM traffic is the two [B,H,Q,K] bias tensors; they are
pre-cast to bf16 on host (34 MB per core, ~95 us at the ~360 GB/s per-core
HBM ceiling) and laid out per-head as [128 k-part, 16 k-tile, 512 q] so each
2 MB DMA moves 16 KB contiguous runs per partition. The score plane is kept
entirely on the PE + Act engines so the slower DVE/GpSimd engines stay off
the critical path and the PE stays busy enough to hold its 2.4 GHz p-state:
  - per score tile the PE computes QK^T (start) and then accumulates both
    bias tiles into the same PSUM bank via identity matmuls (I @ B = B);
  - ScalarE applies exp straight out of PSUM (f32) into bf16;
  - the PE consumes exp(S^T) as the moving operand of the PV matmul;
  - V carries an extra all-ones column per head, so PV emits the softmax
    denominators for free; a tiny [33,128] PE back-transpose restores the
    natural orientation for the per-row normalization and gating.
Q/K are packed 4 heads per 128-partition tile (legal stationary bases
0/32/64/96 at contraction 32), halving K/Q-projection matmul rows.
"""

import numpy as np

B, Q, K, CQ, H, C, CO = 2, 2048, 2048, 256, 8, 32, 256
HC = H * C  # 256
QS = Q // 4  # 512 query rows per core
KT_N = K // 128  # 16 k-tiles
N_CORES = 8
SCALE = float(C) ** -0.5

_CACHED = {}


def _build():
    import concourse.bass as bass
    import concourse.mybir as mybir
    import concourse.tile as tile
    from concourse import bacc
    from concourse.masks import make_identity

    f32 = mybir.dt.float32
    bf16 = mybir.dt.bfloat16
    AF = mybir.ActivationFunctionType
    ALU = mybir.AluOpType

    nc = bacc.Bacc(None, target_bir_lowering=False)

    # activations arrive host-transposed and pre-cast to bf16: [C, rows]
    qxTd = nc.declare_dram_parameter("qxT", [CQ, QS], bf16, isOutput=False)
    kvxTd = nc.declare_dram_parameter("kvxT", [CQ, K], bf16, isOutput=False)
    # biases arrive host-transposed bf16: [H, 128 k-part, 16 k-tile, 512 q]
    b1 = nc.declare_dram_parameter("b1", [H, 128, KT_N, QS], bf16, isOutput=False)
    b2 = nc.declare_dram_parameter("b2", [H, 128, KT_N, QS], bf16, isOutput=False)
    # weights pre-cast to bf16 on host; Wq carries the C**-0.5 scale
    Wq = nc.declare_dram_parameter("Wq", [CQ, HC], bf16, isOutput=False)
    Wk = nc.declare_dram_parameter("Wk", [CQ, HC], bf16, isOutput=False)
    Wv = nc.declare_dram_parameter("Wv", [CQ, HC], bf16, isOutput=False)
    Wg = nc.declare_dram_parameter("Wg", [CQ, HC], bf16, isOutput=False)
    bg = nc.declare_dram_parameter("bg", [HC], f32, isOutput=False)
    Wo = nc.declare_dram_parameter("Wo", [HC, CO], bf16, isOutput=False)
    bo = nc.declare_dram_parameter("bo", [CO], f32, isOutput=False)
    out = nc.declare_dram_parameter("out", [QS, CO], f32, isOutput=True)

    with tile.TileContext(nc) as tc:
        with (
            tc.tile_pool(name="singles", bufs=1) as singles,
            tc.tile_pool(name="stage", bufs=3) as stage,
            tc.tile_pool(name="bias", bufs=1) as biasp,
            tc.tile_pool(name="work", bufs=3) as work,
            tc.tile_pool(name="ework", bufs=4) as ework,
            tc.tile_pool(name="ps", bufs=1, space="PSUM") as psp,
        ):
            ident = singles.tile([128, 128], bf16)
            make_identity(nc, ident)
            identf = singles.tile([128, 128], f32, tag="identf")
            make_identity(nc, identf)

            # bias streams: one 2 MB DMA per (bias, head); b1 on the sync
            # ring, b2 on the scalar HWDGE ring after the setup loads
            # (gpsimd SWDGE blocks its queue on the transfer, so the
            # presums must not share a ring with DMA). 3 heads buffered.
            BIAS_BUFS = 3
            b1_tiles = [None] * H
            b2_tiles = [None] * H

            def issue_b1(h):
                b1t = biasp.tile([128, KT_N, QS], bf16, tag="b1", bufs=BIAS_BUFS)
                nc.sync.dma_start(out=b1t, in_=b1[h])
                b1_tiles[h] = b1t

            def issue_b2(h):
                b2t = biasp.tile([128, KT_N, QS], bf16, tag="b2", bufs=BIAS_BUFS + 1)
                nc.scalar.dma_start(out=b2t, in_=b2[h])
                b2_tiles[h] = b2t

            def issue_bias(h):
                issue_b1(h)
                issue_b2(h)

            # ---- setup loads all on the scalar ring (sync carries the b1
            # stream from t0); kvxT split into 4 column chunks so K-proj
            # starts as soon as the first 384 KB land ----
            for _h in range(BIAS_BUFS):
                issue_b1(_h)
            wbf = {}
            wtile = singles.tile([128, 2, 256], bf16, tag="w_Wk")
            nc.scalar.dma_start(
                out=wtile, in_=Wk[:, :].rearrange("(a p) c -> p a c", p=128)
            )
            wbf["Wk"] = wtile
            kvxT = singles.tile([128, 2, K], bf16, tag="kvxT")
            for kc in range(4):
                ks = slice(kc * 512, (kc + 1) * 512)
                nc.scalar.dma_start(
                    out=kvxT[:, :, kc * 512:(kc + 1) * 512],
                    in_=kvxTd[:, ks].rearrange("(a p) k -> p a k", p=128),
                )
            issue_b2(0)
            wtile = singles.tile([128, 2, 256], bf16, tag="w_Wq")
            nc.scalar.dma_start(
                out=wtile, in_=Wq[:, :].rearrange("(a p) c -> p a c", p=128)
            )
            wbf["Wq"] = wtile
            qxT = singles.tile([128, 2, QS], bf16, tag="qxT")
            nc.scalar.dma_start(
                out=qxT, in_=qxTd[:, :].rearrange("(a p) q -> p a q", p=128)
            )
            issue_b2(1)
            for name, w in (("Wv", Wv), ("Wg", Wg), ("Wo", Wo)):
                wtile = singles.tile([128, 2, 256], bf16, tag=f"w_{name}")
                nc.scalar.dma_start(
                    out=wtile, in_=w[:, :].rearrange("(a p) c -> p a c", p=128)
                )
                wbf[name] = wtile
            bg_bc = singles.tile([128, HC], f32, tag="bg")
            nc.scalar.dma_start(out=bg_bc, in_=bg[:].partition_broadcast(128))
            bo_bc = singles.tile([128, CO], f32, tag="bo")
            nc.scalar.dma_start(out=bo_bc, in_=bo[:].partition_broadcast(128))
            issue_b2(2)

            # Heads packed two per 128-partition tile at bases 0 and 32
            # (legal stationary bases); head h lives at partitions (h%2)*32
            # of pair slot h//2. Projections compute a pair per matmul (M=64).
            QT = singles.tile([128, H // 2, QS], bf16, tag="QT")
            KT = singles.tile([128, H // 2, K], bf16, tag="KT")

            for kc in range(4):
                for j in range(H // 2):
                    cols = slice(j * 64, (j + 1) * 64)
                    ps = psp.tile([128, 512, 1], f32, tag="scores", bufs=4)
                    for ck in range(2):
                        nc.tensor.matmul(
                            ps[:64, :, 0],
                            wbf["Wk"][:, ck, cols],
                            kvxT[:, ck, kc * 512:(kc + 1) * 512],
                            start=(ck == 0),
                            stop=(ck == 1),
                        )
                    nc.vector.tensor_copy(
                        KT[:64, j, kc * 512:(kc + 1) * 512], ps[:64, :, 0]
                    )
            for j in range(H // 2):
                cols = slice(j * 64, (j + 1) * 64)
                ps = psp.tile([128, QS, 1], f32, tag="scores", bufs=4)
                for ck in range(2):
                    nc.tensor.matmul(
                        ps[:64, :, 0],
                        wbf["Wq"][:, ck, cols],
                        qxT[:, ck, :],
                        start=(ck == 0),
                        stop=(ck == 1),
                    )
                nc.vector.tensor_copy(QT[:64, j, :], ps[:64, :, 0])

            # G natural [128q, 4qt, 256hc] f32 = sigmoid(qx @ Wg + bg),
            # computed before any exp so the act table loads only twice.
            Gn = singles.tile([128, 4, HC], f32, tag="Gn")
            for qt in range(4):
                ps = psp.tile([128, HC, 1], f32, tag="scores", bufs=4)
                for ck in range(2):
                    nc.tensor.matmul(
                        ps[:, :, 0],
                        qxT[:, ck, qt * 128:(qt + 1) * 128],
                        wbf["Wg"][:, ck, :],
                        start=(ck == 0),
                        stop=(ck == 1),
                    )
                gt = stage.tile([128, HC], f32, tag="gtmp")
                nc.vector.tensor_add(gt, ps[:, :, 0], bg_bc)
                nc.scalar.activation(Gn[:, qt, :], gt, AF.Sigmoid)

            # V natural [128kr, 16kt, 8h, 33] bf16; per head 32 V columns
            # plus an all-ones column so the PV matmul emits softmax
            # denominators for free in output column 32. The Vn memset also
            # delays the gpsimd ring's first b2 prefetch so the setup loads
            # get full DMA bandwidth for the first ~10 us.
            Vn = singles.tile([128, KT_N, H, 33], bf16, tag="Vn")
            nc.gpsimd.memset(Vn, 1.0)
            for kt in range(KT_N):
                ps = psp.tile([128, H, C, 1], f32, tag="scores", bufs=4)
                for ck in range(2):
                    nc.tensor.matmul(
                        ps[:, :, :, 0],
                        kvxT[:, ck, kt * 128:(kt + 1) * 128],
                        wbf["Wv"][:, ck, :],
                        start=(ck == 0),
                        stop=(ck == 1),
                    )
                nc.vector.tensor_copy(Vn[:, kt, :, 0:C], ps[:, :, :, 0])

            # ---- main attention loops (transposed orientation) ----
            # Global software pipeline over (head, k-tile-pair) units, with
            # gpsimd pre-summing b1+b2 two k-tiles ahead of the PE. All
            # score-plane arithmetic stays on the PE (QK^T start, bias
            # accumulate via identity matmuls, stop) so the PE remains the
            # system bottleneck and holds its fast clock; ScalarE applies
            # exp straight out of PSUM; PV consumes exp(S^T) two tiles
            # behind, so the PE never waits on the Act engine. Per head,
            # k-tiles 0-11 use pre-summed bias (3 PE matmuls), k-tiles 12-15
            # accumulate both bias tiles directly (4 PE matmuls) while the
            # gpsimd works ahead on the next head.
            O_all = singles.tile([128, 4, HC], f32, tag="O_all")
            PROD_LEAD = 2
            NPAIR = KT_N // 2
            units = [(h, p) for h in range(H) for p in range(NPAIR)]
            bs_tiles = {}
            o_tiles = {}
            norm_pend = []
            pv_pend = []

            def emit_normalize():
                ph, oT_sb = norm_pend.pop(0)
                phcol = ph * 32
                for qt in range(4):
                    on_ps = psp.tile([128, C + 1, 1], f32, tag="onat", bufs=1)
                    nc.tensor.transpose(
                        on_ps[:, :, 0],
                        oT_sb[:, qt * 128:(qt + 1) * 128],
                        identf[:33, :33],
                    )
                    rinv = work.tile([128, 1], f32, tag="rinv")
                    nc.vector.reciprocal(rinv, on_ps[:, C:C + 1, 0])
                    nc.vector.tensor_scalar_mul(
                        O_all[:, qt, phcol:phcol + 32], on_ps[:, :C, 0], rinv
                    )

            def prod(h, p):
                if p < 6:
                    B1h, B2h = b1_tiles[h], b2_tiles[h]
                    kt2 = 2 * p
                    Bs2 = ework.tile([128, 2, QS], bf16, tag="bsum", bufs=4)
                    nc.gpsimd.tensor_tensor(
                        Bs2, B1h[:, kt2:kt2 + 2, :], B2h[:, kt2:kt2 + 2, :],
                        ALU.add,
                    )
                    bs_tiles[(h, p)] = Bs2

            def emit_pv():
                ph, pkt, pet = pv_pend.pop(0)
                nc.tensor.matmul(
                    o_tiles[ph][:, :, 0],
                    Vn[:, pkt, ph, :],
                    pet,
                    start=(pkt == 0),
                    stop=(pkt == KT_N - 1),
                )
                if pkt == KT_N - 1:
                    oT_sb = work.tile([33, QS], f32, tag="oT", bufs=3)
                    nc.vector.tensor_copy(oT_sb, o_tiles.pop(ph)[:, :, 0])
                    norm_pend.append((ph, oT_sb))

            def cons(h, p):
                base = (h % 2) * 32
                hsl = slice(base, base + 32)
                g = h // 2
                B1h, B2h = b1_tiles[h], b2_tiles[h]
                if p == 0:
                    o_ps = psp.tile([33, QS, 1], f32, tag="o_acc", bufs=2)
                    o_tiles[h] = o_ps
                Bs2 = bs_tiles.pop((h, p), None)
                for kt in (2 * p, 2 * p + 1):
                    s_ps = psp.tile([128, QS, 1], f32, tag="scores", bufs=4)
                    nc.tensor.matmul(
                        s_ps[:, :, 0],
                        KT[hsl, g, kt * 128:(kt + 1) * 128],
                        QT[hsl, g, :],
                        start=True,
                        stop=False,
                    )
                    if Bs2 is not None:
                        nc.tensor.matmul(
                            s_ps[:, :, 0], ident, Bs2[:, kt - 2 * p, :],
                            start=False, stop=True,
                        )
                    else:
                        nc.tensor.matmul(
                            s_ps[:, :, 0], ident, B1h[:, kt, :],
                            start=False, stop=False,
                        )
                        nc.tensor.matmul(
                            s_ps[:, :, 0], ident, B2h[:, kt, :],
                            start=False, stop=True,
                        )
                    et_sb = ework.tile([128, QS], bf16, tag="et")
                    nc.scalar.activation(et_sb, s_ps[:, :, 0], AF.Exp)
                    pv_pend.append((h, kt, et_sb))
                    if len(pv_pend) > 2:
                        emit_pv()
                if p == 2 and norm_pend:
                    emit_normalize()
                if p == NPAIR - 1 and h + BIAS_BUFS < H:
                    # issued here (not in prod) so the scalar ring's
                    # slot-reuse wait is already satisfied and never blocks
                    # the exp stream
                    issue_bias(h + BIAS_BUFS)

            for i in range(len(units) + PROD_LEAD):
                if i < len(units):
                    prod(*units[i])
                if i >= PROD_LEAD:
                    cons(*units[i - PROD_LEAD])
            while pv_pend:
                emit_pv()
            while norm_pend:
                emit_normalize()

            # ---- gating + output projection ----
            for qt in range(4):
                og = stage.tile([128, HC], bf16, tag="og")
                nc.vector.tensor_mul(og, O_all[:, qt, :], Gn[:, qt, :])
                ogt_ps = psp.tile([128, 2, 128], bf16, tag="et_ps", bufs=1)
                for hcc in range(2):
                    nc.tensor.transpose(
                        ogt_ps[:, hcc, :], og[:, hcc * 128:(hcc + 1) * 128], ident
                    )
                ogt = stage.tile([128, 2, 128], bf16, tag="ogt")
                nc.vector.tensor_copy(ogt, ogt_ps)
                f_ps = psp.tile([128, CO, 1], f32, tag="scores", bufs=4)
                for hcc in range(2):
                    nc.tensor.matmul(
                        f_ps[:, :, 0],
                        ogt[:, hcc, :],
                        wbf["Wo"][:, hcc, :],
                        start=(hcc == 0),
                        stop=(hcc == 1),
                    )
                o_sb = stage.tile([128, CO], f32, tag="o_out")
                nc.vector.tensor_add(o_sb, f_ps[:, :, 0], bo_bc)
                nc.sync.dma_start(out=out[qt * 128:(qt + 1) * 128, :], in_=o_sb)

    nc.compile()
    return nc


def _get_nc():
    if "nc" not in _CACHED:
        _CACHED["nc"] = _build()
    return _CACHED["nc"]


def kernel(**inputs):
    from concourse.bass_utils import run_bass_kernel_spmd

    import ml_dtypes

    bf = ml_dtypes.bfloat16
    nc = _get_nc()
    inp = {k: np.asarray(v, dtype=np.float32) for k, v in inputs.items()}
    wq_b = (inp["Wq"] * SCALE).astype(bf)
    wk_b = inp["Wk"].astype(bf)
    wv_b = inp["Wv"].astype(bf)
    wg_b = inp["Wg"].astype(bf)
    wo_b = inp["Wo"].astype(bf)

    def bias_layout(x):
        # [H, QS, K] -> [H, 128 k-part, 16 k-tile, QS] bf16
        x = x.reshape(H, QS, KT_N, 128).transpose(0, 3, 2, 1)
        return np.ascontiguousarray(x).astype(bf)

    in_maps = []
    for c in range(N_CORES):
        b, qi = c // 4, c % 4
        q0 = qi * QS
        in_maps.append({
            "qxT": np.ascontiguousarray(inp["q_x"][b, q0:q0 + QS, :].T).astype(bf),
            "kvxT": np.ascontiguousarray(inp["kv_x"][b].T).astype(bf),
            "b1": bias_layout(inp["bias1"][b, :, q0:q0 + QS, :]),
            "b2": bias_layout(inp["bias2"][b, :, q0:q0 + QS, :]),
            "Wq": wq_b, "Wk": wk_b, "Wv": wv_b, "Wg": wg_b,
            "bg": inp["bg"], "Wo": wo_b, "bo": inp["bo"],
        })
    res = run_bass_kernel_spmd(nc, in_maps, core_ids=list(range(N_CORES)))
    outa = np.empty((B, Q, CO), np.float32)
    for c in range(N_CORES):
        b, qi = c // 4, c % 4
        outa[b, qi * QS:(qi + 1) * QS, :] = res.results[c]["out"]
    return outa


# revision 21
# speedup vs baseline: 1.0164x; 1.0164x over previous
"""Trainium2 8-core kernel for biased-attention with sigmoid gating.

Reference computation (per batch b):
  q = heads(q_x @ Wq) * C**-0.5 ; k = heads(kv_x @ Wk) ; v = heads(kv_x @ Wv)
  a = softmax(q k^T + bias1 + bias2, axis=-1)
  o = (a @ v) gated by sigmoid(q_x @ Wg + bg), then @ Wo + bo

Shapes: B=2, Q=K=2048, CQ=CK=CV=256, H=8, C=32, CO=256.

Sharding: 8 cores = 2 batches x 4 query-quarters (512 rows each). Each core
computes all 8 heads for its rows; no cross-core communication is needed.

Design notes:
- The dominant HBM traffic is the two [B,H,Q,K] bias tensors; they are
  pre-cast to bf16 on host (34 MB per core) and laid out per-head as
  [128 k-part, 16 k-tile, 512 q] so each 2 MB DMA moves 16 KB contiguous
  runs per partition. b1 streams on the sync ring, b2 on the gpsimd ring
  initially and the scalar ring in steady state.
- Scores are produced directly in the transposed [k, q] orientation so the
  exp output feeds the PV matmul as the moving operand with no transposes.
- gpsimd/DVE pre-sum b1+b2 two k-tiles per op, running two tile-pairs ahead
  of the PE (global software pipeline over all (head, pair) units).
- Per head, k-tiles 0-8 accumulate the pre-summed bias into the QK PSUM
  bank via an identity matmul (exp reads PSUM); k-tiles 9-15 instead let
  the DVE add the bias while draining PSUM to SBUF (exp reads SBUF).
  This splits the score-plane elementwise work across PE/DVE/Act so the
  PE stays the system bottleneck (holding its fast clock) at the fewest
  total PE matmuls.
- V carries an extra all-ones column per head, so PV emits the softmax
  denominators for free; a tiny [33,128] PE back-transpose restores the
  natural orientation for the per-row normalization, deferred into the
  next head's stream so the PE never waits on the DVE.
"""

import numpy as np

B, Q, K, CQ, H, C, CO = 2, 2048, 2048, 256, 8, 32, 256
HC = H * C  # 256
QS = Q // 4  # 512 query rows per core
KT_N = K // 128  # 16 k-tiles
N_CORES = 8
SCALE = float(C) ** -0.5

_CACHED = {}


def _build():
    import concourse.bass as bass
    import concourse.mybir as mybir
    import concourse.tile as tile
    from concourse import bacc
    from concourse.masks import make_identity

    f32 = mybir.dt.float32
    bf16 = mybir.dt.bfloat16
    AF = mybir.ActivationFunctionType
    ALU = mybir.AluOpType

    nc = bacc.Bacc(None, target_bir_lowering=False)

    qxTd = nc.declare_dram_parameter("qxT", [CQ, QS], bf16, isOutput=False)
    kvxTd = nc.declare_dram_parameter("kvxT", [CQ, K], bf16, isOutput=False)
    b1 = nc.declare_dram_parameter("b1", [H, 128, KT_N, QS], bf16, isOutput=False)
    b2 = nc.declare_dram_parameter("b2", [H, 128, KT_N, QS], bf16, isOutput=False)
    Wq = nc.declare_dram_parameter("Wq", [CQ, HC], bf16, isOutput=False)
    Wk = nc.declare_dram_parameter("Wk", [CQ, HC], bf16, isOutput=False)
    Wv = nc.declare_dram_parameter("Wv", [CQ, HC], bf16, isOutput=False)
    Wg = nc.declare_dram_parameter("Wg", [CQ, HC], bf16, isOutput=False)
    bg = nc.declare_dram_parameter("bg", [HC], f32, isOutput=False)
    Wo = nc.declare_dram_parameter("Wo", [HC, CO], bf16, isOutput=False)
    bo = nc.declare_dram_parameter("bo", [CO], f32, isOutput=False)
    out = nc.declare_dram_parameter("out", [QS, CO], f32, isOutput=True)

    with tile.TileContext(nc) as tc:
        with (
            tc.tile_pool(name="singles", bufs=1) as singles,
            tc.tile_pool(name="stage", bufs=3) as stage,
            tc.tile_pool(name="bias", bufs=1) as biasp,
            tc.tile_pool(name="work", bufs=3) as work,
            tc.tile_pool(name="ework", bufs=4) as ework,
            tc.tile_pool(name="ps", bufs=1, space="PSUM") as psp,
        ):
            ident = singles.tile([128, 128], bf16)
            make_identity(nc, ident)
            identf = singles.tile([128, 128], f32, tag="identf")
            make_identity(nc, identf)

            # bias stream plumbing. One 2 MB DMA per (bias, head).
            BIAS_BUFS = 3
            b1_tiles = [None] * H
            b2_tiles = [None] * H

            def issue_b1(h):
                b1t = biasp.tile([128, KT_N, QS], bf16, tag="b1", bufs=BIAS_BUFS)
                nc.sync.dma_start(out=b1t, in_=b1[h])
                b1_tiles[h] = b1t

            def issue_b2(h, ring):
                b2t = biasp.tile(
                    [128, KT_N, QS], bf16, tag="b2", bufs=BIAS_BUFS + 1
                )
                ring.dma_start(out=b2t, in_=b2[h])
                b2_tiles[h] = b2t

            # ---- setup loads: scalar ring carries the K-proj critical path
            # (Wk, kvxT halves); sync carries the rest, then the b1 stream.
            wbf = {}
            wtile = singles.tile([128, 2, 256], bf16, tag="w_Wk")
            nc.scalar.dma_start(
                out=wtile, in_=Wk[:, :].rearrange("(a p) c -> p a c", p=128)
            )
            wbf["Wk"] = wtile
            kvxT = singles.tile([128, 2, K], bf16, tag="kvxT")
            for half in range(2):
                nc.scalar.dma_start(
                    out=kvxT[:, :, half * (K // 2):(half + 1) * (K // 2)],
                    in_=kvxTd[:, half * (K // 2):(half + 1) * (K // 2)].rearrange(
                        "(a p) k -> p a k", p=128
                    ),
                )
            wtile = singles.tile([128, 2, 256], bf16, tag="w_Wq")
            nc.sync.dma_start(
                out=wtile, in_=Wq[:, :].rearrange("(a p) c -> p a c", p=128)
            )
            wbf["Wq"] = wtile
            qxT = singles.tile([128, 2, QS], bf16, tag="qxT")
            nc.sync.dma_start(
                out=qxT, in_=qxTd[:, :].rearrange("(a p) q -> p a q", p=128)
            )
            for name, w in (("Wv", Wv), ("Wg", Wg), ("Wo", Wo)):
                wtile = singles.tile([128, 2, 256], bf16, tag=f"w_{name}")
                nc.sync.dma_start(
                    out=wtile, in_=w[:, :].rearrange("(a p) c -> p a c", p=128)
                )
                wbf[name] = wtile
            bg_bc = singles.tile([128, HC], f32, tag="bg")
            nc.sync.dma_start(out=bg_bc, in_=bg[:].partition_broadcast(128))
            bo_bc = singles.tile([128, CO], f32, tag="bo")
            nc.sync.dma_start(out=bo_bc, in_=bo[:].partition_broadcast(128))

            # Heads packed two per 128-partition tile at bases 0 and 32;
            # head h lives at partitions (h%2)*32 of pair slot h//2.
            QT = singles.tile([128, H // 2, QS], bf16, tag="QT")
            KT = singles.tile([128, H // 2, K], bf16, tag="KT")

            for j in range(H // 2):
                cols = slice(j * 64, (j + 1) * 64)
                for kc in range(4):
                    ps = psp.tile([128, 512, 1], f32, tag="scores", bufs=4)
                    for ck in range(2):
                        nc.tensor.matmul(
                            ps[:64, :, 0],
                            wbf["Wk"][:, ck, cols],
                            kvxT[:, ck, kc * 512:(kc + 1) * 512],
                            start=(ck == 0),
                            stop=(ck == 1),
                        )
                    nc.vector.tensor_copy(
                        KT[:64, j, kc * 512:(kc + 1) * 512], ps[:64, :, 0]
                    )
                ps = psp.tile([128, QS, 1], f32, tag="scores", bufs=4)
                for ck in range(2):
                    nc.tensor.matmul(
                        ps[:64, :, 0],
                        wbf["Wq"][:, ck, cols],
                        qxT[:, ck, :],
                        start=(ck == 0),
                        stop=(ck == 1),
                    )
                nc.vector.tensor_copy(QT[:64, j, :], ps[:64, :, 0])

            # G natural [128q, 4qt, 256hc] f32 = sigmoid(qx @ Wg + bg),
            # computed before any exp so the act table loads only twice.
            Gn = singles.tile([128, 4, HC], f32, tag="Gn")
            for qt in range(4):
                ps = psp.tile([128, HC, 1], f32, tag="scores", bufs=4)
                for ck in range(2):
                    nc.tensor.matmul(
                        ps[:, :, 0],
                        qxT[:, ck, qt * 128:(qt + 1) * 128],
                        wbf["Wg"][:, ck, :],
                        start=(ck == 0),
                        stop=(ck == 1),
                    )
                gt = stage.tile([128, HC], f32, tag="gtmp")
                nc.vector.tensor_add(gt, ps[:, :, 0], bg_bc)
                nc.scalar.activation(Gn[:, qt, :], gt, AF.Sigmoid)

            # V natural [128kr, 16kt, 8h, 33] bf16; per head 32 V columns
            # plus an all-ones column so the PV matmul emits softmax
            # denominators for free in output column 32. The Vn memset also
            # delays the gpsimd ring's first b2 prefetch so the setup loads
            # get full DMA bandwidth early.
            Vn = singles.tile([128, KT_N, H, 33], bf16, tag="Vn")
            nc.gpsimd.memset(Vn, 1.0)
            for _h in range(BIAS_BUFS):
                issue_b1(_h)
                issue_b2(_h, nc.gpsimd)
            for kt in range(KT_N):
                ps = psp.tile([128, H, C, 1], f32, tag="scores", bufs=4)
                for ck in range(2):
                    nc.tensor.matmul(
                        ps[:, :, :, 0],
                        kvxT[:, ck, kt * 128:(kt + 1) * 128],
                        wbf["Wv"][:, ck, :],
                        start=(ck == 0),
                        stop=(ck == 1),
                    )
                nc.vector.tensor_copy(Vn[:, kt, :, 0:C], ps[:, :, :, 0])

            # ---- main attention loops (transposed orientation) ----
            O_all = singles.tile([128, 4, HC], f32, tag="O_all")
            PROD_LEAD = 2
            NPAIR = KT_N // 2
            PE_ACC_TILES = 9  # k-tiles 0..8 on the PE-accumulate path
            units = [(h, p) for h in range(H) for p in range(NPAIR)]
            bs_tiles = {}
            o_tiles = {}
            norm_pend = []
            pv_pend = []

            def emit_normalize():
                ph, oT_sb = norm_pend.pop(0)
                phcol = ph * 32
                for qt in range(4):
                    on_ps = psp.tile([128, C + 1, 1], f32, tag="onat", bufs=1)
                    nc.tensor.transpose(
                        on_ps[:, :, 0],
                        oT_sb[:, qt * 128:(qt + 1) * 128],
                        identf[:33, :33],
                    )
                    rinv = work.tile([128, 1], f32, tag="rinv")
                    nc.vector.reciprocal(rinv, on_ps[:, C:C + 1, 0])
                    nc.vector.tensor_scalar_mul(
                        O_all[:, qt, phcol:phcol + 32], on_ps[:, :C, 0], rinv
                    )

            def prod(h, p):
                # pre-sum b1+b2 for k-tiles 2p, 2p+1; pairs 0-4 on gpsimd,
                # 5-7 on the DVE (bf16 runs 2x there)
                B1h, B2h = b1_tiles[h], b2_tiles[h]
                kt2 = 2 * p
                eng = nc.gpsimd if p < 5 else nc.vector
                Bs2 = ework.tile([128, 2, QS], bf16, tag="bsum", bufs=4)
                eng.tensor_tensor(
                    Bs2, B1h[:, kt2:kt2 + 2, :], B2h[:, kt2:kt2 + 2, :],
                    ALU.add,
                )
                bs_tiles[(h, p)] = Bs2

            def emit_pv():
                ph, pkt, pet = pv_pend.pop(0)
                nc.tensor.matmul(
                    o_tiles[ph][:, :, 0],
                    Vn[:, pkt, ph, :],
                    pet,
                    start=(pkt == 0),
                    stop=(pkt == KT_N - 1),
                )
                if pkt == KT_N - 1:
                    oT_sb = work.tile([33, QS], f32, tag="oT", bufs=3)
                    nc.vector.tensor_copy(oT_sb, o_tiles.pop(ph)[:, :, 0])
                    norm_pend.append((ph, oT_sb))

            def cons(h, p):
                base = (h % 2) * 32
                hsl = slice(base, base + 32)
                g = h // 2
                if p == 0:
                    o_ps = psp.tile([33, QS, 1], f32, tag="o_acc", bufs=2)
                    o_tiles[h] = o_ps
                Bs2 = bs_tiles.pop((h, p))
                for kt in (2 * p, 2 * p + 1):
                    pe_acc = kt < PE_ACC_TILES
                    s_ps = psp.tile([128, QS, 1], f32, tag="scores", bufs=4)
                    nc.tensor.matmul(
                        s_ps[:, :, 0],
                        KT[hsl, g, kt * 128:(kt + 1) * 128],
                        QT[hsl, g, :],
                        start=True,
                        stop=not pe_acc,
                    )
                    if pe_acc:
                        nc.tensor.matmul(
                            s_ps[:, :, 0], ident, Bs2[:, kt - 2 * p, :],
                            start=False, stop=True,
                        )
                        et_in = s_ps[:, :, 0]
                    else:
                        t_sb = ework.tile([128, QS], bf16, tag="tsum", bufs=4)
                        nc.vector.tensor_tensor(
                            t_sb, s_ps[:, :, 0], Bs2[:, kt - 2 * p, :],
                            ALU.add,
                        )
                        et_in = t_sb
                    et_sb = ework.tile([128, QS], bf16, tag="et")
                    nc.scalar.activation(et_sb, et_in, AF.Exp)
                    pv_pend.append((h, kt, et_sb))
                    if len(pv_pend) > 2:
                        emit_pv()
                if p == 2 and norm_pend:
                    emit_normalize()
                if p == NPAIR - 1 and h + BIAS_BUFS < H:
                    # issued here so the ring's slot-reuse wait is already
                    # satisfied and never blocks the exp stream
                    issue_b1(h + BIAS_BUFS)
                    issue_b2(h + BIAS_BUFS, nc.scalar)

            for i in range(len(units) + PROD_LEAD):
                if i < len(units):
                    prod(*units[i])
                if i >= PROD_LEAD:
                    cons(*units[i - PROD_LEAD])
            while pv_pend:
                emit_pv()
            while norm_pend:
                emit_normalize()

            # ---- gating + output projection ----
            for qt in range(4):
                og = stage.tile([128, HC], bf16, tag="og")
                nc.vector.tensor_mul(og, O_all[:, qt, :], Gn[:, qt, :])
                ogt_ps = psp.tile([128, 2, 128], bf16, tag="et_ps", bufs=1)
                for hcc in range(2):
                    nc.tensor.transpose(
                        ogt_ps[:, hcc, :], og[:, hcc * 128:(hcc + 1) * 128], ident
                    )
                ogt = stage.tile([128, 2, 128], bf16, tag="ogt")
                nc.vector.tensor_copy(ogt, ogt_ps)
                f_ps = psp.tile([128, CO, 1], f32, tag="scores", bufs=4)
                for hcc in range(2):
                    nc.tensor.matmul(
                        f_ps[:, :, 0],
                        ogt[:, hcc, :],
                        wbf["Wo"][:, hcc, :],
                        start=(hcc == 0),
                        stop=(hcc == 1),
                    )
                o_sb = stage.tile([128, CO], f32, tag="o_out")
                nc.vector.tensor_add(o_sb, f_ps[:, :, 0], bo_bc)
                nc.sync.dma_start(out=out[qt * 128:(qt + 1) * 128, :], in_=o_sb)

    nc.compile()
    return nc


def _get_nc():
    if "nc" not in _CACHED:
        _CACHED["nc"] = _build()
    return _CACHED["nc"]


def kernel(**inputs):
    from concourse.bass_utils import run_bass_kernel_spmd

    import ml_dtypes

    bf = ml_dtypes.bfloat16
    nc = _get_nc()
    inp = {k: np.asarray(v, dtype=np.float32) for k, v in inputs.items()}
    wq_b = (inp["Wq"] * SCALE).astype(bf)
    wk_b = inp["Wk"].astype(bf)
    wv_b = inp["Wv"].astype(bf)
    wg_b = inp["Wg"].astype(bf)
    wo_b = inp["Wo"].astype(bf)

    def bias_layout(x):
        # [H, QS, K] -> [H, 128 k-part, 16 k-tile, QS] bf16
        x = x.reshape(H, QS, KT_N, 128).transpose(0, 3, 2, 1)
        return np.ascontiguousarray(x).astype(bf)

    in_maps = []
    for c in range(N_CORES):
        b, qi = c // 4, c % 4
        q0 = qi * QS
        in_maps.append({
            "qxT": np.ascontiguousarray(inp["q_x"][b, q0:q0 + QS, :].T).astype(bf),
            "kvxT": np.ascontiguousarray(inp["kv_x"][b].T).astype(bf),
            "b1": bias_layout(inp["bias1"][b, :, q0:q0 + QS, :]),
            "b2": bias_layout(inp["bias2"][b, :, q0:q0 + QS, :]),
            "Wq": wq_b, "Wk": wk_b, "Wv": wv_b, "Wg": wg_b,
            "bg": inp["bg"], "Wo": wo_b, "bo": inp["bo"],
        })
    res = run_bass_kernel_spmd(nc, in_maps, core_ids=list(range(N_CORES)))
    outa = np.empty((B, Q, CO), np.float32)
    for c in range(N_CORES):
        b, qi = c // 4, c % 4
        outa[b, qi * QS:(qi + 1) * QS, :] = res.results[c]["out"]
    return outa
